# revision 1
# baseline (speedup 1.0000x reference)
"""Trainium2 Bass kernel for nn_MeanDegConv (gnn_message_passing) on 8 NeuronCores.

Self-contained: imports the Bass/Tile stack from /opt/trn_rl_repo (part of the
container environment) and hardcodes all shapes/sharding for the problem.
"""
import sys
for _p in ('/opt/trn_rl_repo',):
    if _p not in sys.path:
        sys.path.insert(0, _p)

import numpy as np

import concourse.bass as bass
import concourse.mybir as mybir
import concourse.tile as tile
import concourse.bacc as bacc
from concourse.bass_utils import run_bass_kernel_spmd

N, E, NNZ, D = 50000, 10000, 1000000, 128
C = 8
EPC, VPC = E // C, N // C          # 1250 edges, 6250 vertices per core
NWIN_E = (EPC + 127) // 128        # 10
NWIN_V = (VPC + 127) // 128        # 49
EP = NWIN_E * 128                  # 1280 padded edge slots per core
VP = NWIN_V * 128                  # 6272 padded vertex slots per core
CHUNK = 8192                       # gather indices per dma_gather call
TPC = CHUNK // 128                 # 64 tiles per chunk
SPLIT = 32768                      # int16 index limit for the X table

F32 = mybir.dt.float32
BF16 = mybir.dt.bfloat16
I16 = mybir.dt.int16


def _pack_idx16(idx32: np.ndarray) -> np.ndarray:
    """[L] int32 -> [128, L/16] int16 in the dma_gather wrap layout."""
    L = len(idx32)
    assert L % 16 == 0
    a = idx32.astype(np.int16).reshape(L // 16, 16).T  # [16, L/16]
    return np.ascontiguousarray(np.tile(a, (8, 1)))    # [128, L/16]


def _pad_to(arr, L, fill):
    out = np.full(L, fill, arr.dtype)
    out[:len(arr)] = arr
    return out


def _build_stream(per_win_idx, per_win_lidx, tiles_per_win):
    """Concatenate per-window (idx, lidx) entries, padding each window to
    tiles_per_win[w]*128 entries (idx pad 0, lidx pad -1). Returns idx
    [Lt], lidx [Lt] with Lt = sum(tiles)*128 padded to CHUNK multiple."""
    idx_parts, lidx_parts = [], []
    for w, T in enumerate(tiles_per_win):
        L = T * 128
        idx_parts.append(_pad_to(per_win_idx[w], L, 0))
        lidx_parts.append(_pad_to(per_win_lidx[w], L, -1.0))
    idx = np.concatenate(idx_parts) if idx_parts else np.zeros(0, np.int32)
    lidx = np.concatenate(lidx_parts) if lidx_parts else np.zeros(0, np.float32)
    Lt = ((len(idx) + CHUNK - 1) // CHUNK) * CHUNK
    return _pad_to(idx, Lt, 0), _pad_to(lidx, Lt, -1.0)


def prepare(inputs, mm_dt=F32):
    """Host-side preprocessing: consts, per-core streams, schedule."""
    X = np.asarray(inputs["X"], np.float32)
    X0 = np.asarray(inputs["X0"], np.float32)
    v = np.asarray(inputs["vertex"]).astype(np.int64)
    e = np.asarray(inputs["edges"]).astype(np.int64)
    W1_w = np.asarray(inputs["W1_w"], np.float32); W1_b = np.asarray(inputs["W1_b"], np.float32)
    W2_w = np.asarray(inputs["W2_w"], np.float32); W2_b = np.asarray(inputs["W2_b"], np.float32)
    W3_w1 = np.asarray(inputs["W3_w1"], np.float32); W3_b1 = np.asarray(inputs["W3_b1"], np.float32)
    W3_w2 = np.asarray(inputs["W3_w2"], np.float32); W3_b2 = np.asarray(inputs["W3_b2"], np.float32)

    deg_e = np.bincount(e, minlength=E).astype(np.float32)
    deg_v = np.bincount(v, minlength=N).astype(np.float32)

    # ---- folded weight matrices (float64 for accuracy, cast at the end)
    W2a = W2_w[:D].astype(np.float64); W2b1 = W2_w[D:2*D].astype(np.float64)
    w2b_log = W2_w[2*D].astype(np.float64)
    R1 = W3_w1[:D].astype(np.float64); R2 = W3_w1[D:2*D].astype(np.float64)
    R3 = W3_w1[2*D:3*D].astype(np.float64); r4 = W3_w1[3*D].astype(np.float64)
    W2bR = W2b1 @ R1
    K1 = (W1_w.astype(np.float64) @ W2bR).astype(np.float32)
    k2 = (w2b_log @ R1).astype(np.float32)
    c1 = (W1_b.astype(np.float64) @ W2bR).astype(np.float32)
    MX = (W2a @ R1 + R2).astype(np.float32)
    MX0 = R3.astype(np.float32)
    c0 = (W2_b.astype(np.float64) @ R1 + W3_b1).astype(np.float32)

    consts = {
        "iota": np.ascontiguousarray(
            np.tile(np.arange(128, dtype=np.float32), (128, 1))),
        "K1": K1,
        "K2": np.ascontiguousarray(np.stack([k2, c1])),            # [2,128]
        "MX": MX, "MX0": MX0,
        "RC2": np.ascontiguousarray(np.stack([r4.astype(np.float32), c0])),  # [2,128]
        "W3w2": W3_w2,
        "b2row": W3_b2.reshape(1, D),
        "ones1": np.ones((1, 128), np.float32),
        "Xtab": X,                                                  # gather table
    }

    # ---- stage-1: per (core, window, half) incidence lists
    core1 = (e // EPC).astype(np.int64)          # owning core by edge range
    win1 = ((e % EPC) // 128).astype(np.int64)   # window within core
    lidx1 = ((e % EPC) % 128).astype(np.float32) # slot within window
    half1 = (v >= SPLIT).astype(np.int64)

    # bucket sort indices by (core, window, half)
    key1 = (core1 * NWIN_E + win1) * 2 + half1
    order1 = np.argsort(key1, kind="stable")
    ks = key1[order1]
    bounds1 = np.searchsorted(ks, np.arange(C * NWIN_E * 2 + 1))

    def seg1(c, w, h):
        b = (c * NWIN_E + w) * 2 + h
        return order1[bounds1[b]:bounds1[b + 1]]

    cnt1 = np.diff(bounds1).reshape(C, NWIN_E, 2)
    TA = [int(np.ceil(cnt1[:, w, 0].max() / 128)) for w in range(NWIN_E)]
    TB = [int(np.ceil(cnt1[:, w, 1].max() / 128)) for w in range(NWIN_E)]

    # ---- stage-2: per (core, window) lists, indices are padded xe row ids
    core2 = (v // VPC).astype(np.int64)
    win2 = ((v % VPC) // 128).astype(np.int64)
    lidx2 = ((v % VPC) % 128).astype(np.float32)
    rowid2 = (e // EPC) * EP + (e % EPC)         # padded row in xe_all

    key2 = core2 * NWIN_V + win2
    order2 = np.argsort(key2, kind="stable")
    ks2 = key2[order2]
    bounds2 = np.searchsorted(ks2, np.arange(C * NWIN_V + 1))

    def seg2(c, w):
        b = c * NWIN_V + w
        return order2[bounds2[b]:bounds2[b + 1]]

    cnt2 = np.diff(bounds2).reshape(C, NWIN_V)
    T2 = [int(np.ceil(cnt2[:, w].max() / 128)) for w in range(NWIN_V)]

    sched = {"TA": TA, "TB": TB, "T2": T2, "mm_dt": mm_dt}

    # ---- per-core input maps
    in_maps = []
    log_deg_e = np.log(deg_e); log_deg_v = np.log(deg_v)
    for c in range(C):
        # stage-1 streams
        idxA = [v[seg1(c, w, 0)].astype(np.int32) for w in range(NWIN_E)]
        lidA = [lidx1[seg1(c, w, 0)] for w in range(NWIN_E)]
        idxB = [(v[seg1(c, w, 1)] - SPLIT).astype(np.int32) for w in range(NWIN_E)]
        lidB = [lidx1[seg1(c, w, 1)] for w in range(NWIN_E)]
        sA_idx, sA_lid = _build_stream(idxA, lidA, TA)
        sB_idx, sB_lid = _build_stream(idxB, lidB, TB)
        # stage-2 stream
        idx2 = [rowid2[seg2(c, w)].astype(np.int32) for w in range(NWIN_V)]
        lid2 = [lidx2[seg2(c, w)] for w in range(NWIN_V)]
        s2_idx, s2_lid = _build_stream(idx2, lid2, T2)

        # per-core edge aux (padded slots get deg=1, log=0)
        de = np.ones(EP, np.float32); de[:EPC] = deg_e[c*EPC:(c+1)*EPC]
        le = np.zeros(EP, np.float32); le[:EPC] = log_deg_e[c*EPC:(c+1)*EPC]
        auxe = np.ascontiguousarray(np.stack([de * le, de]))        # [2, EP]
        invdeg_e_col = np.ascontiguousarray(
            (1.0 / de).reshape(NWIN_E, 128).T)                      # [128, NWIN_E]

        dv = np.ones(VP, np.float32); dv[:VPC] = deg_v[c*VPC:(c+1)*VPC]
        lv = np.zeros(VP, np.float32); lv[:VPC] = log_deg_v[c*VPC:(c+1)*VPC]
        auxv = np.ascontiguousarray(np.stack([lv, np.ones(VP, np.float32)]))  # [2, VP]
        invdeg_bc = np.ascontiguousarray(
            np.tile(1.0 / dv, (128, 1)))                            # [128, VP]

        Xp = np.zeros((VP, D), np.float32); Xp[:VPC] = X[c*VPC:(c+1)*VPC]
        X0p = np.zeros((VP, D), np.float32); X0p[:VPC] = X0[c*VPC:(c+1)*VPC]

        m = dict(consts)
        m.update({
            "idxA": _pack_idx16(sA_idx), "lidA": np.ascontiguousarray(
                sA_lid.reshape(-1, 128).T),
            "idxB": _pack_idx16(sB_idx), "lidB": np.ascontiguousarray(
                sB_lid.reshape(-1, 128).T),
            "idx2": _pack_idx16(s2_idx), "lid2": np.ascontiguousarray(
                s2_lid.reshape(-1, 128).T),
            "auxe": auxe, "invdeg_e_col": invdeg_e_col,
            "auxv": auxv, "invdeg_bc": invdeg_bc,
            "XT": np.ascontiguousarray(Xp.T), "X0T": np.ascontiguousarray(X0p.T),
        })
        in_maps.append(m)
    return in_maps, sched


def build(in_map0, sched, mode="full"):
    """Build the SPMD Bass program. in_map0 supplies shapes."""
    TA, TB, T2 = sched["TA"], sched["TB"], sched["T2"]
    mm_dt = sched["mm_dt"]
    nc = bacc.Bacc(None)

    def param(name, dt=F32):
        arr = in_map0[name]
        return nc.declare_dram_parameter(name, list(arr.shape), dt, isOutput=False)

    Xtab_d = param("Xtab")
    iota_d = param("iota"); K1_d = param("K1"); K2_d = param("K2")
    MX_d = param("MX"); MX0_d = param("MX0"); RC2_d = param("RC2")
    W3w2_d = param("W3w2"); b2row_d = param("b2row"); ones1_d = param("ones1")
    idxA_d = param("idxA", I16); lidA_d = param("lidA")
    idxB_d = param("idxB", I16); lidB_d = param("lidB")
    idx2_d = param("idx2", I16); lid2_d = param("lid2")
    auxe_d = param("auxe"); invde_d = param("invdeg_e_col")
    auxv_d = param("auxv"); invbc_d = param("invdeg_bc")
    XT_d = param("XT"); X0T_d = param("X0T")
    out_d = nc.declare_dram_parameter("out", [VP, D], F32, isOutput=True)

    LA = in_map0["idxA"].shape[1] * 16
    LB = in_map0["idxB"].shape[1] * 16
    L2 = in_map0["idx2"].shape[1] * 16
    nchA, nchB, nch2 = LA // CHUNK, LB // CHUNK, L2 // CHUNK

    with tile.TileContext(nc) as tc:
        with (
            tc.tile_pool(name="const", bufs=1) as cp,
            tc.tile_pool(name="stream", bufs=1) as sp,
            tc.tile_pool(name="g", bufs=2) as gp,
            tc.tile_pool(name="work", bufs=3) as wp,
            tc.tile_pool(name="acc", bufs=1) as accp,
            tc.tile_pool(name="psS", bufs=1, space="PSUM") as psS,
            tc.tile_pool(name="psXE", bufs=1, space="PSUM") as psXE,
            tc.tile_pool(name="psT", bufs=2, space="PSUM") as psT,
            tc.tile_pool(name="psR", bufs=2, space="PSUM") as psR,
            tc.tile_pool(name="psO", bufs=1, space="PSUM") as psO,
            tc.tile_pool(name="dram", bufs=1, space="DRAM") as dp,
        ):
            # ---- load constants / streams
            def load(pool, dram_ap, name, dt=F32, eng=None):
                t = pool.tile(list(dram_ap.shape), dt, name=name, tag=name)
                (eng or nc.sync).dma_start(t[:], dram_ap[:])
                return t

            iota_t = load(cp, iota_d, "iota")
            K1_t = load(cp, K1_d, "K1"); K2_t = load(cp, K2_d, "K2")
            MX_t = load(cp, MX_d, "MX"); MX0_t = load(cp, MX0_d, "MX0")
            RC2_t = load(cp, RC2_d, "RC2")
            W3w2_t = load(cp, W3w2_d, "W3w2"); b2row_t = load(cp, b2row_d, "b2row")
            ones1_t = load(cp, ones1_d, "ones1")
            auxe_t = load(cp, auxe_d, "auxe"); invde_t = load(cp, invde_d, "invde")
            auxv_t = load(cp, auxv_d, "auxv")
            idxA_t = load(sp, idxA_d, "idxA", I16); lidA_t = load(sp, lidA_d, "lidA")
            idxB_t = load(sp, idxB_d, "idxB", I16); lidB_t = load(sp, lidB_d, "lidB")
            idx2_t = load(sp, idx2_d, "idx2", I16); lid2_t = load(sp, lid2_d, "lid2")

            xe_local = dp.tile([EP, D], F32)
            xe_all = dp.tile([C * EP, D], F32, addr_space="Shared")
            xe_tab = dp.tile([C * EP, D], F32)

            sA_sb = accp.tile([128, EP], F32)   # S^T accumulated (pass A, then +B)

            # ================= stage 1 =================
            def gather_pass(idx_t, lid_t, nch, Ts, in_ap, consume):
                """Issue chunked gathers; `consume(w, t, g_slice, lid_col)`
                is called per (window, tile)."""
                chunks = []
                for ci in range(nch):
                    g = gp.tile([128, TPC, D], F32, tag="g")
                    nc.gpsimd.dma_gather(
                        out_ap=g[:],
                        in_ap=in_ap,
                        idxs_ap=idx_t[:, ci * (CHUNK // 16):(ci + 1) * (CHUNK // 16)],
                        num_idxs=CHUNK,
                        num_idxs_reg=CHUNK,
                        single_packet=False,
                        elem_size=D,
                    )
                    chunks.append(g)
                tc_ctr = 0
                for w, T in enumerate(Ts):
                    for t in range(T):
                        g = chunks[tc_ctr // TPC]
                        slot = tc_ctr % TPC
                        consume(w, t, T, g[:, slot, :], lid_t[:, tc_ctr:tc_ctr + 1])
                        tc_ctr += 1

            # pass A: accumulate into psum, flush to sA_sb
            stateA = {}
            def consumeA(w, t, T, g_sl, lid_col):
                if t == 0:
                    stateA["ps"] = psS.tile([128, 128], F32, tag="s1", name="psA")
                p = wp.tile([128, 128], mm_dt, tag="p1")
                nc.vector.tensor_scalar(
                    out=p[:], in0=iota_t[:], scalar1=lid_col, scalar2=None,
                    op0=mybir.AluOpType.is_equal)
                g_mm = g_sl.bitcast(mm_dt) if mm_dt != F32 else g_sl
                nc.tensor.matmul(stateA["ps"][:], g_mm, p[:],
                                 start=(t == 0), stop=(t == T - 1))
                if t == T - 1:
                    nc.scalar.copy(sA_sb[:, w * 128:(w + 1) * 128], stateA["ps"][:])

            gather_pass(idxA_t, lidA_t, nchA, TA, Xtab_d[0:SPLIT, :], consumeA)

            # pass B: accumulate into psum, add into sA_sb
            stateB = {}
            def consumeB(w, t, T, g_sl, lid_col):
                if t == 0:
                    stateB["ps"] = psS.tile([128, 128], F32, tag="s1", name="psB")
                p = wp.tile([128, 128], mm_dt, tag="p1")
                nc.vector.tensor_scalar(
                    out=p[:], in0=iota_t[:], scalar1=lid_col, scalar2=None,
                    op0=mybir.AluOpType.is_equal)
                g_mm = g_sl.bitcast(mm_dt) if mm_dt != F32 else g_sl
                nc.tensor.matmul(stateB["ps"][:], g_mm, p[:],
                                 start=(t == 0), stop=(t == T - 1))
                if t == T - 1:
                    nc.vector.tensor_tensor(
                        out=sA_sb[:, w * 128:(w + 1) * 128],
                        in0=sA_sb[:, w * 128:(w + 1) * 128],
                        in1=stateB["ps"][:], op=mybir.AluOpType.add)

            gather_pass(idxB_t, lidB_t, nchB, TB, Xtab_d[SPLIT:N, :], consumeB)

            # xe_hat per window: psum = S^T.T@K1 + auxe.T@K2, scale by 1/deg
            for w in range(NWIN_E):
                ps = psXE.tile([128, 128], F32, tag="xe")
                nc.tensor.matmul(ps[:], sA_sb[:, w * 128:(w + 1) * 128], K1_t[:],
                                 start=True, stop=False)
                nc.tensor.matmul(ps[:], auxe_t[:, w * 128:(w + 1) * 128], K2_t[:],
                                 start=False, stop=True)
                xe_sb = wp.tile([128, D], F32, tag="xe_sb")
                nc.scalar.activation(
                    out=xe_sb[:], in_=ps[:],
                    func=mybir.ActivationFunctionType.Copy,
                    scale=invde_t[:, w:w + 1])
                nc.sync.dma_start(xe_local[w * 128:(w + 1) * 128, :], xe_sb[:])

            if mode == "s1":
                # dump xe_local rows into out for validation
                for w in range(NWIN_E):
                    xe_rd = wp.tile([128, D], F32, tag="xe_rd", name="xe_rd")
                    nc.sync.dma_start(xe_rd[:], xe_local[w * 128:(w + 1) * 128, :])
                    nc.sync.dma_start(out_d[w * 128:(w + 1) * 128, :], xe_rd[:])
            if mode in ("s1ag", "full"):
                # ================= allgather =================
                nc.gpsimd.collective_compute(
                    "AllGather", mybir.AluOpType.bypass,
                    replica_groups=[list(range(C))],
                    ins=[xe_local.opt()], outs=[xe_all.opt()])
                nc.sync.dma_start(xe_tab[:], xe_all[:])
            if mode == "s1ag":
                for w in range(NWIN_V):
                    xe_rd = wp.tile([128, D], F32, tag="xe_rd", name="xe_rd")
                    nc.sync.dma_start(xe_rd[:], xe_tab[w * 128:(w + 1) * 128, :])
                    nc.sync.dma_start(out_d[w * 128:(w + 1) * 128, :], xe_rd[:])
            if mode == "full":
                # ================= stage 2 =================
                state2 = {}
                def consume2(w, t, T, g_sl, lid_col):
                    if t == 0:
                        state2["ps"] = psT.tile([128, 128], F32, tag="t3", name="psT2")
                    p = wp.tile([128, 128], mm_dt, tag="p2")
                    nc.vector.tensor_scalar(
                        out=p[:], in0=iota_t[:], scalar1=lid_col, scalar2=None,
                        op0=mybir.AluOpType.is_equal)
                    g_mm = g_sl.bitcast(mm_dt) if mm_dt != F32 else g_sl
                    nc.tensor.matmul(state2["ps"][:], g_mm, p[:],
                                     start=(t == 0), stop=(t == T - 1))
                    if t == T - 1:
                        finish_window(w, state2["ps"])

                def finish_window(w, psT_tile):
                    sl = slice(w * 128, (w + 1) * 128)
                    xt = wp.tile([128, 128], F32, tag="xt", name="xt")
                    x0t = wp.tile([128, 128], F32, tag="x0t", name="x0t")
                    invbc = wp.tile([128, 128], F32, tag="invbc", name="invbc")
                    nc.sync.dma_start(xt[:], XT_d[:, sl])
                    nc.sync.dma_start(x0t[:], X0T_d[:, sl])
                    nc.sync.dma_start(invbc[:], invbc_d[:, sl])
                    psr = psR.tile([128, 128], F32, tag="r", name="psr")
                    nc.tensor.matmul(psr[:], MX_t[:], xt[:], start=True, stop=False)
                    nc.tensor.matmul(psr[:], MX0_t[:], x0t[:], start=False, stop=False)
                    nc.tensor.matmul(psr[:], RC2_t[:], auxv_t[:, sl], start=False, stop=True)
                    pre = wp.tile([128, 128], F32, tag="pre", name="pre")
                    nc.vector.tensor_tensor(out=pre[:], in0=psT_tile[:],
                                            in1=invbc[:], op=mybir.AluOpType.mult)
                    nc.vector.tensor_tensor(out=pre[:], in0=pre[:], in1=psr[:],
                                            op=mybir.AluOpType.add)
                    relu = wp.tile([128, 128], F32, tag="relu", name="relu")
                    nc.scalar.activation(out=relu[:], in_=pre[:],
                                         func=mybir.ActivationFunctionType.Relu)
                    pso = psO.tile([128, 128], F32, tag="o", name="pso")
                    nc.tensor.matmul(pso[:], relu[:], W3w2_t[:], start=True, stop=False)
                    nc.tensor.matmul(pso[:], ones1_t[:], b2row_t[:], start=False, stop=True)
                    o_sb = wp.tile([128, D], F32, tag="o_sb", name="o_sb")
                    nc.scalar.copy(o_sb[:], pso[:])
                    nc.sync.dma_start(out_d[w * 128:(w + 1) * 128, :], o_sb[:])

                gather_pass(idx2_t, lid2_t, nch2, T2, xe_tab[:], consume2)

    nc.finalize()
    return nc


def run(trace=False, mode="full", **inputs):
    in_maps, sched = prepare(inputs)
    nc = build(in_maps[0], sched, mode=mode)
    res = run_bass_kernel_spmd(nc, in_maps, list(range(C)), trace=trace)
    out = np.concatenate([res.results[c]["out"][:VPC] for c in range(C)], axis=0)
    return out, res


def kernel(**inputs):
    """Harness entry point: full inputs in, full [N, D] float32 output."""
    out, _res = run(trace=False, mode="full", **inputs)
    return out.astype(np.float32)



# revision 6
# speedup vs baseline: 3.0320x; 3.0320x over previous
"""Trainium2 Bass kernel for nn_MeanDegConv (gnn_message_passing) on 8 NeuronCores.

Round-based design: incidences are laid out as (window, round, slot) grids so
segment sums become PSUM-accumulating identity/diag matmuls (no per-tile
one-hot builds on the vector engine). Gather tables are bf16 (halved DMA
bytes) and gather descriptor generation rotates across SWDGE queues.

Self-contained: imports the Bass/Tile stack from /opt/trn_rl_repo (part of the
container environment) and hardcodes all shapes/sharding for the problem.
"""
import sys
for _p in ('/opt/trn_rl_repo',):
    if _p not in sys.path:
        sys.path.insert(0, _p)

import numpy as np

import concourse.bass as bass
import concourse.mybir as mybir
import concourse.tile as tile
import concourse.bacc as bacc
from concourse.bass_utils import run_bass_kernel_spmd

N, E, NNZ, D = 50000, 10000, 1000000, 128
C = 8
EPC, VPC = E // C, N // C          # 1250 edges, 6250 vertices per core
NWIN_E = (EPC + 127) // 128        # 10
NWIN_V = (VPC + 127) // 128        # 49
EP = NWIN_E * 128                  # 1280 padded edge slots per core
VP = NWIN_V * 128                  # 6272 padded vertex slots per core
CHUNK = 4096                       # gather indices per dma_gather call
TPC = CHUNK // 128                 # tiles per chunk
NQ = 4                             # SWDGE queues to rotate desc-gen across

SPLA = 32767                       # XA covers vertices [0, 32767); zero row at 32767
XB_BASE = N - SPLA                 # 17233; XB covers [17233, 50000); zero row at 32767
XTAB_ROWS = SPLA + 1               # 32768 rows per split table
XE_ROWS = C * EP                   # 10240 real xe rows
XE_ZERO = XE_ROWS                  # zero row index in xe_tab
XE_TAB_ROWS = XE_ROWS + 16         # padded alloc

F32 = mybir.dt.float32
BF16 = mybir.dt.bfloat16
I16 = mybir.dt.int16
BFNP = mybir.dt.np(BF16)


def _pack_idx16(idx32: np.ndarray) -> np.ndarray:
    """[L] int32 -> [128, L/16] int16 in the dma_gather wrap layout."""
    L = len(idx32)
    assert L % 16 == 0
    a = idx32.astype(np.int16).reshape(L // 16, 16).T  # [16, L/16]
    return np.ascontiguousarray(np.tile(a, (8, 1)))    # [128, L/16]


def _padlen(L):
    return ((L + CHUNK - 1) // CHUNK) * CHUNK


def prepare(inputs):
    X = np.asarray(inputs["X"], np.float32)
    X0 = np.asarray(inputs["X0"], np.float32)
    v = np.asarray(inputs["vertex"]).astype(np.int64)
    e = np.asarray(inputs["edges"]).astype(np.int64)
    W1_w = np.asarray(inputs["W1_w"], np.float32); W1_b = np.asarray(inputs["W1_b"], np.float32)
    W2_w = np.asarray(inputs["W2_w"], np.float32); W2_b = np.asarray(inputs["W2_b"], np.float32)
    W3_w1 = np.asarray(inputs["W3_w1"], np.float32); W3_b1 = np.asarray(inputs["W3_b1"], np.float32)
    W3_w2 = np.asarray(inputs["W3_w2"], np.float32); W3_b2 = np.asarray(inputs["W3_b2"], np.float32)

    deg_e = np.bincount(e, minlength=E)
    deg_v = np.bincount(v, minlength=N)

    # ---- folded weight matrices (float64 for accuracy, cast at the end)
    W2a = W2_w[:D].astype(np.float64); W2b1 = W2_w[D:2*D].astype(np.float64)
    w2b_log = W2_w[2*D].astype(np.float64)
    R1 = W3_w1[:D].astype(np.float64); R2 = W3_w1[D:2*D].astype(np.float64)
    R3 = W3_w1[2*D:3*D].astype(np.float64); r4 = W3_w1[3*D].astype(np.float64)
    W2bR = W2b1 @ R1
    K1 = (W1_w.astype(np.float64) @ W2bR).astype(np.float32)
    k2 = (w2b_log @ R1).astype(np.float32)
    c1 = (W1_b.astype(np.float64) @ W2bR).astype(np.float32)
    MX = (W2a @ R1 + R2).astype(np.float32)
    MX0 = R3.astype(np.float32)
    c0 = (W2_b.astype(np.float64) @ R1 + W3_b1).astype(np.float32)

    # ---- permutations: sort by degree desc, deal round-robin to cores
    eperm = np.argsort(-deg_e, kind="stable")
    e_core = np.empty(E, np.int64); e_pos = np.empty(E, np.int64)
    e_core[eperm] = np.arange(E) % C
    e_pos[eperm] = np.arange(E) // C
    vperm = np.argsort(-deg_v, kind="stable")
    v_core = np.empty(N, np.int64); v_pos = np.empty(N, np.int64)
    v_core[vperm] = np.arange(N) % C
    v_pos[vperm] = np.arange(N) // C

    # ---- stage 1: A/B balanced split per edge
    cls = np.where(v < XB_BASE, 0, np.where(v >= SPLA, 2, 1))
    nAf = np.bincount(e[cls == 0], minlength=E)
    nBf = np.bincount(e[cls == 2], minlength=E)
    cntA = np.clip((deg_e + 1) // 2, nAf, deg_e - nBf)

    cA = np.zeros((C, EP), np.int64); cB = np.zeros((C, EP), np.int64)
    cA[e_core, e_pos] = cntA
    cB[e_core, e_pos] = deg_e - cntA
    RA = cA.reshape(C, NWIN_E, 128).max(axis=(0, 2))
    RB = cB.reshape(C, NWIN_E, 128).max(axis=(0, 2))
    LA = int(RA.sum()) * 128
    LB = int(RB.sum()) * 128

    # order incidences by (edge, class): forced-A, middles, forced-B
    oinc = np.argsort(e * 4 + cls, kind="stable")
    e_s = e[oinc]; v_s = v[oinc]
    starts = np.searchsorted(e_s, np.arange(E))
    rank = np.arange(NNZ) - starts[e_s]
    isA = rank < cntA[e_s]
    ecore1 = e_core[e_s]; epos1 = e_pos[e_s]
    w1 = epos1 // 128; s1 = epos1 % 128
    offA = np.zeros(NWIN_E, np.int64); offA[1:] = np.cumsum(RA)[:-1]
    offB = np.zeros(NWIN_E, np.int64); offB[1:] = np.cumsum(RB)[:-1]
    posA = (offA[w1] + rank) * 128 + s1
    posB = (offB[w1] + (rank - cntA[e_s])) * 128 + s1

    # ---- stage 2 rounds
    cV = np.zeros((C, VP), np.int64)
    cV[v_core, v_pos] = deg_v
    R2r = cV.reshape(C, NWIN_V, 128).max(axis=(0, 2))
    L2 = int(R2r.sum()) * 128

    rowid_of_e = e_core * EP + e_pos
    o2 = np.argsort(v, kind="stable")
    v_s2 = v[o2]; e_s2 = e[o2]
    starts2 = np.searchsorted(v_s2, np.arange(N))
    rank2 = np.arange(NNZ) - starts2[v_s2]
    vcore2 = v_core[v_s2]; vpos2 = v_pos[v_s2]
    w2 = vpos2 // 128; s2w = vpos2 % 128
    off2 = np.zeros(NWIN_V, np.int64); off2[1:] = np.cumsum(R2r)[:-1]
    pos2 = (off2[w2] + rank2) * 128 + s2w
    val2 = rowid_of_e[e_s2]

    sched = {"RA": [int(x) for x in RA], "RB": [int(x) for x in RB],
             "R2": [int(x) for x in R2r],
             "LA": _padlen(LA), "LB": _padlen(LB), "L2": _padlen(L2)}

    # ---- shared consts
    Xb = X.astype(BFNP)
    XA = np.zeros((XTAB_ROWS, D), BFNP); XA[:SPLA] = Xb[:SPLA]
    XBt = np.zeros((XTAB_ROWS, D), BFNP); XBt[:N - XB_BASE] = Xb[XB_BASE:]
    deg_ef = deg_e.astype(np.float32); log_deg_e = np.log(deg_ef)
    deg_vf = deg_v.astype(np.float32); log_deg_v = np.log(deg_vf)

    consts = {
        "XA": XA, "XB": XBt,
        "iota": np.tile(np.arange(128, dtype=np.float32), (128, 1)).astype(BFNP),
        "iotacol": np.arange(128, dtype=np.float32).reshape(128, 1),
        "K1": K1.astype(BFNP),
        "K2": np.ascontiguousarray(np.stack([k2, c1])),
        "MX": MX, "MX0": MX0,
        "RC2": np.ascontiguousarray(np.stack([r4.astype(np.float32), c0])),
        "W3w2": W3_w2.astype(BFNP),
        "b2row": W3_b2.reshape(1, D),
        "ones1": np.ones((1, 128), np.float32),
    }

    edge_at = np.full((C, EP), -1, np.int64)
    edge_at[e_core, e_pos] = np.arange(E)
    vert_at = np.full((C, VP), -1, np.int64)
    vert_at[v_core, v_pos] = np.arange(N)

    in_maps = []
    for c in range(C):
        mA = (ecore1 == c) & isA
        mB = (ecore1 == c) & (~isA)
        sA = np.full(sched["LA"], SPLA, np.int32)
        sA[posA[mA]] = v_s[mA]
        sB = np.full(sched["LB"], SPLA, np.int32)
        sB[posB[mB]] = v_s[mB] - XB_BASE

        m2 = vcore2 == c
        s2 = np.full(sched["L2"], XE_ZERO, np.int32)
        s2[pos2[m2]] = val2[m2]

        de = np.ones(EP, np.float32); le = np.zeros(EP, np.float32)
        msk = edge_at[c] >= 0
        de[msk] = deg_ef[edge_at[c][msk]]
        le[msk] = log_deg_e[edge_at[c][msk]]
        auxe = np.ascontiguousarray(np.stack([de * le, de]))
        invde_col = np.ascontiguousarray((1.0 / de).reshape(NWIN_E, 128).T)

        dv = np.ones(VP, np.float32); lv = np.zeros(VP, np.float32)
        vm = vert_at[c] >= 0
        dv[vm] = deg_vf[vert_at[c][vm]]
        lv[vm] = log_deg_v[vert_at[c][vm]]
        auxv = np.ascontiguousarray(np.stack([lv, np.ones(VP, np.float32)]))
        invdv_col = np.ascontiguousarray((1.0 / dv).reshape(NWIN_V, 128).T)

        Xp = np.zeros((VP, D), np.float32); X0p = np.zeros((VP, D), np.float32)
        Xp[vm] = X[vert_at[c][vm]]
        X0p[vm] = X0[vert_at[c][vm]]

        m = dict(consts)
        m.update({
            "idxA": _pack_idx16(sA), "idxB": _pack_idx16(sB),
            "idx2": _pack_idx16(s2),
            "auxe": auxe, "invde_col": invde_col,
            "auxv": auxv, "invdv_col": invdv_col,
            "XT": np.ascontiguousarray(Xp.T), "X0T": np.ascontiguousarray(X0p.T),
        })
        in_maps.append(m)
    unperm = {"v_core": v_core, "v_pos": v_pos}
    return in_maps, sched, unperm


def build(in_map0, sched, nq=NQ):
    RA, RB, R2 = sched["RA"], sched["RB"], sched["R2"]
    nc = bacc.Bacc(None, num_swdge_queues=nq)

    def param(name, dt=F32):
        arr = in_map0[name]
        return nc.declare_dram_parameter(name, list(arr.shape), dt, isOutput=False)

    XA_d = param("XA", BF16); XB_d = param("XB", BF16)
    iota_d = param("iota", BF16); iotacol_d = param("iotacol")
    K1_d = param("K1", BF16); K2_d = param("K2")
    MX_d = param("MX"); MX0_d = param("MX0"); RC2_d = param("RC2")
    W3w2_d = param("W3w2", BF16); b2row_d = param("b2row"); ones1_d = param("ones1")
    idxA_d = param("idxA", I16); idxB_d = param("idxB", I16); idx2_d = param("idx2", I16)
    auxe_d = param("auxe"); invde_d = param("invde_col")
    auxv_d = param("auxv"); invdv_d = param("invdv_col")
    XT_d = param("XT"); X0T_d = param("X0T")
    out_d = nc.declare_dram_parameter("out", [VP, D], F32, isOutput=True)

    # Rotate desc-gen across SWDGE queues 1..3: their Q7 core-pairs generate
    # descriptors off the Pool engine's critical path (queue 0 blocks ~32us).
    qrot = [1, 2, 3] if nq == 4 else list(range(nq))
    qctr = [0]

    def next_q():
        q = qrot[qctr[0] % len(qrot)]
        qctr[0] += 1
        return q

    with tile.TileContext(nc) as tc:
        with (
            tc.tile_pool(name="const", bufs=1) as cp,
            tc.tile_pool(name="stream", bufs=1) as sp,
            tc.tile_pool(name="g", bufs=8) as gp,
            tc.tile_pool(name="work", bufs=3) as wp,
            tc.tile_pool(name="psA", bufs=2, space="PSUM") as psA,
            tc.tile_pool(name="psT", bufs=2, space="PSUM") as psT,
            tc.tile_pool(name="psF", bufs=2, space="PSUM") as psF,
            tc.tile_pool(name="dram", bufs=1, space="DRAM") as dp,
        ):
            def load(pool, dram_ap, name, dt=F32):
                t = pool.tile(list(dram_ap.shape), dt, name=name, tag=name)
                nc.sync.dma_start(t[:], dram_ap[:])
                return t

            iota_t = load(cp, iota_d, "iota", BF16)
            iotacol_t = load(cp, iotacol_d, "iotacol")
            K1_t = load(cp, K1_d, "K1", BF16); K2_t = load(cp, K2_d, "K2")
            MX_t = load(cp, MX_d, "MX"); MX0_t = load(cp, MX0_d, "MX0")
            RC2_t = load(cp, RC2_d, "RC2")
            W3w2_t = load(cp, W3w2_d, "W3w2", BF16)
            b2row_t = load(cp, b2row_d, "b2row"); ones1_t = load(cp, ones1_d, "ones1")
            auxe_t = load(cp, auxe_d, "auxe"); invde_t = load(cp, invde_d, "invde")
            auxv_t = load(cp, auxv_d, "auxv"); invdv_t = load(cp, invdv_d, "invdv")
            XT_t = load(cp, XT_d, "XT"); X0T_t = load(cp, X0T_d, "X0T")
            idxA_t = load(sp, idxA_d, "idxA", I16)
            idxB_t = load(sp, idxB_d, "idxB", I16)
            idx2_t = load(sp, idx2_d, "idx2", I16)

            # identity (bf16): Id[s, j] = (iota[s, j] == s)
            Id_t = cp.tile([128, 128], BF16, name="Id", tag="Id")
            nc.vector.tensor_scalar(
                out=Id_t[:], in0=iota_t[:], scalar1=iotacol_t[:, 0:1],
                scalar2=None, op0=mybir.AluOpType.is_equal)
            # per-vertex-window diag(1/deg_v) bf16
            diag_t = cp.tile([128, NWIN_V, 128], BF16, name="diag", tag="diag")
            for w in range(NWIN_V):
                nc.vector.tensor_scalar(
                    out=diag_t[:, w, :], in0=iota_t[:], scalar1=iotacol_t[:, 0:1],
                    scalar2=invdv_t[:, w:w + 1], op0=mybir.AluOpType.is_equal,
                    op1=mybir.AluOpType.mult)

            xe_local = dp.tile([EP, D], BF16)
            xe_all = dp.tile([C * EP, D], BF16, addr_space="Shared")
            xe_tab = dp.tile([XE_TAB_ROWS, D], BF16)

            # ---- lazy chunked gathers: one shared SBUF ring, issue on demand
            chunks = {}

            def get_tile(stream, pos, idx_t, in_ap):
                lst = chunks.setdefault(stream, [])
                ci = pos // CHUNK
                while len(lst) <= ci:
                    k = len(lst)
                    g = gp.tile([128, TPC, D], BF16, tag="g", name=f"g{stream}{k}")
                    nc.gpsimd.dma_gather(
                        out_ap=g[:], in_ap=in_ap,
                        idxs_ap=idx_t[:, k * (CHUNK // 16):(k + 1) * (CHUNK // 16)],
                        num_idxs=CHUNK, num_idxs_reg=CHUNK,
                        single_packet=False, elem_size=D, queue_num=next_q())
                    lst.append(g)
                return lst[ci][:, (pos % CHUNK) // 128, :]

            # ============ stage 1 ============
            pA = [0]; pB = [0]
            for w in range(NWIN_E):
                ra, rb = RA[w], RB[w]
                ps = psA.tile([128, 128], F32, tag="acc", name=f"psS{w}")
                for r in range(ra):
                    t = get_tile("A", pA[0], idxA_t, XA_d[:]); pA[0] += 128
                    nc.tensor.matmul(ps[:], Id_t[:], t, start=(r == 0), stop=False)
                for r in range(rb):
                    t = get_tile("B", pB[0], idxB_t, XB_d[:]); pB[0] += 128
                    nc.tensor.matmul(ps[:], Id_t[:], t,
                                     start=False, stop=(r == rb - 1))
                s_sb = wp.tile([128, 128], BF16, tag="s_sb", name=f"s_sb{w}")
                nc.scalar.copy(s_sb[:], ps[:])
                pst = psT.tile([128, 128], F32, tag="t", name=f"psT{w}")
                nc.tensor.matmul(pst[:], s_sb[:], Id_t[:], start=True, stop=True)
                st_sb = wp.tile([128, 128], BF16, tag="st_sb", name=f"st_sb{w}")
                nc.scalar.copy(st_sb[:], pst[:])
                pxe = psF.tile([128, 128], F32, tag="fin", name=f"psXE{w}")
                nc.tensor.matmul(pxe[:], st_sb[:], K1_t[:], start=True, stop=False)
                nc.tensor.matmul(pxe[:], auxe_t[:, w * 128:(w + 1) * 128], K2_t[:],
                                 start=False, stop=True)
                xe_sb = wp.tile([128, D], BF16, tag="xe_sb", name=f"xe_sb{w}")
                nc.scalar.activation(
                    out=xe_sb[:], in_=pxe[:],
                    func=mybir.ActivationFunctionType.Copy,
                    scale=invde_t[:, w:w + 1])
                nc.sync.dma_start(xe_local[w * 128:(w + 1) * 128, :], xe_sb[:])

            # ============ allgather + zero row ============
            nc.gpsimd.collective_compute(
                "AllGather", mybir.AluOpType.bypass,
                replica_groups=[list(range(C))],
                ins=[xe_local.opt()], outs=[xe_all.opt()])
            nc.sync.dma_start(xe_tab[0:C * EP, :], xe_all[:])
            zrow = wp.tile([16, D], BF16, tag="zrow", name="zrow")
            nc.vector.memset(zrow[:], 0.0)
            nc.sync.dma_start(xe_tab[XE_ZERO:XE_ZERO + 16, :], zrow[:])

            # ============ stage 2 ============
            p2 = [0]
            for w in range(NWIN_V):
                sl = slice(w * 128, (w + 1) * 128)
                r2 = R2[w]
                pre = psA.tile([128, 128], F32, tag="acc", name=f"psP{w}")
                for r in range(r2):
                    t = get_tile("2", p2[0], idx2_t, xe_tab[:]); p2[0] += 128
                    nc.tensor.matmul(pre[:], diag_t[:, w, :], t,
                                     start=(r == 0), stop=False)
                nc.tensor.matmul(pre[:], XT_t[:, sl], MX_t[:], start=False, stop=False)
                nc.tensor.matmul(pre[:], X0T_t[:, sl], MX0_t[:], start=False, stop=False)
                nc.tensor.matmul(pre[:], auxv_t[:, sl], RC2_t[:], start=False, stop=True)
                relu_sb = wp.tile([128, 128], BF16, tag="relu", name=f"relu{w}")
                nc.scalar.activation(out=relu_sb[:], in_=pre[:],
                                     func=mybir.ActivationFunctionType.Relu)
                prt = psT.tile([128, 128], F32, tag="t", name=f"psRT{w}")
                nc.tensor.matmul(prt[:], relu_sb[:], Id_t[:], start=True, stop=True)
                rt_sb = wp.tile([128, 128], BF16, tag="rt", name=f"rt{w}")
                nc.scalar.copy(rt_sb[:], prt[:])
                pso = psF.tile([128, 128], F32, tag="fin", name=f"psO{w}")
                nc.tensor.matmul(pso[:], rt_sb[:], W3w2_t[:], start=True, stop=False)
                nc.tensor.matmul(pso[:], ones1_t[:], b2row_t[:], start=False, stop=True)
                o_sb = wp.tile([128, D], F32, tag="o_sb", name=f"o_sb{w}")
                nc.scalar.copy(o_sb[:], pso[:])
                nc.sync.dma_start(out_d[sl, :], o_sb[:])

    nc.finalize()
    return nc


def run(trace=False, nq=NQ, **inputs):
    in_maps, sched, unperm = prepare(inputs)
    nc = build(in_maps[0], sched, nq=nq)
    res = run_bass_kernel_spmd(nc, in_maps, list(range(C)), trace=trace)
    out = np.empty((N, D), np.float32)
    v_core, v_pos = unperm["v_core"], unperm["v_pos"]
    for c in range(C):
        oc = res.results[c]["out"]          # [VP, D]
        mask = v_core == c
        out[mask] = oc[v_pos[mask]]
    return out, res


def kernel(**inputs):
    """Harness entry point: full inputs in, full [N, D] float32 output."""
    out, _res = run(trace=False, **inputs)
    return out.astype(np.float32)
